# revision 2
# baseline (speedup 1.0000x reference)
"""Channel-attention block kernel for Trainium2 (8 NeuronCores, SPMD) — v2.

Reference (per batch b):
    q = x[b]                       # [C=512, N=4096]  (N = H*W)
    aff = q @ q.T                  # [512, 512]
    attn = softmax(rowmax(aff) - aff, axis=-1)
         = exp(rowmin(aff) - aff) / rowsum(...)
    out[b] = gamma * (attn @ x[b]) + x[b]

v2 over the baseline (183us):
  - y stored fp16 on device (host casts back to f32): halves output DMA.
  - aff(b0) uses PSUM banks 0-3, aff(b1) banks 4-7; MM2 po tiles reuse
    the other batch's banks.  Removes every WAR stall between batches.
  - qT XBAR transposes are one whole-C transfer per 1024-col group
    (first group split finer so MM1 starts early); q loads for MM2 are
    emitted AFTER the transposes so they don't starve MM1 on the
    serialized DMA path.
  - recon/softmax/attnT emission interleaved into the next phase's PE
    stream so the PE never idles between MM1(b0)/MM1(b1)/MM2(b0)/MM2(b1).
"""

import numpy as np

import concourse.bacc as bacc
import concourse.tile as tile
from concourse import mybir
from concourse.bass_utils import run_bass_kernel_spmd
from concourse.masks import make_identity

B, C, H, W = 16, 512, 64, 64
N = H * W            # 4096
NCORES = 8
BPC = B // NCORES    # batches per core
CP = C // 128        # 4 channel blocks
KP = N // 128        # 32 n-chunks
KG = 4               # qT transpose groups per batch
KPG = KP // KG       # 8 k-chunks per group

f32 = mybir.dt.float32
f16 = mybir.dt.float16

# lower-triangle blocks reconstructed from their upper mirrors, ordered
# so softmax rows complete in order 1,2,3 (row 0 needs no recon)
TRI = [(1, 0), (2, 0), (2, 1), (3, 0), (3, 1), (3, 2)]


def _build_body(nc, tc, x, gamma, y):
    pools = {}

    def pool(name, bufs, space="SBUF"):
        pools[name] = tc.alloc_tile_pool(name=name, bufs=bufs, space=space)
        return pools[name]

    q_p = pool("q", BPC)                  # [128, N] f16 per channel block
    qt_p = pool("qt", BPC)                # [128, KPG, 512] f16 qT groups
    tri_p = pool("tri", 6)                # [128, 128] f32 recon staging
    attn_p = pool("attn", 2 * CP)         # [128, C] f16
    attnT_p = pool("attnT", 2)            # [128, CP, 512] f16
    outsb_p = pool("outsb", 6)            # [128, 2048] f16
    small_p = pool("small", 8)            # [128, 1] f32
    const_p = pool("const", 1)
    ps_p = pool("ps", 1, space="PSUM")    # tags a0..a7 -> the 8 banks

    def aff_tile(t, name):
        return ps_p.tile([128, C], f32, tag=f"a{t}", name=name)

    # qT via DMA XBAR straight from DRAM: group g transposes
    # x[b, :, 1024g:1024(g+1)] ([512c, 1024n]) into qt[g] = [128n, 8k, 512c].
    # XBAR transposes stay on the SP ring only (concurrent transposes on
    # both HWDGE rings corrupt nondeterministically on HW).
    def alloc_qt(b):
        return [
            qt_p.tile([128, KPG, 512], f16, tag=f"qt{g}", name=f"qt{b}{g}")
            for g in range(KG)
        ]

    def emit_qT(b, qtg, g, splits=((0, KPG),)):
        for (k0, k1) in splits:
            nc.sync.dma_start(
                out=qtg[g][:, k0:k1, :],
                in_=x[b, :, 1024 * g + 128 * k0:1024 * g + 128 * k1],
                transpose=True,
            )

    def xbar_qT(b, fine=False):
        # fine=True (critical-path batch 0): half-group transfers so the
        # chunk-arrival curve stays ahead of MM1's consumption rate
        qtg = alloc_qt(b)
        for g in range(KG):
            if fine:
                splits = ((0, 2), (2, 4), (4, 8)) if g == 0 else ((0, 4), (4, 8))
            else:
                splits = ((0, KPG),)
            emit_qT(b, qtg, g, splits)
        return qtg

    def alloc_q(b):
        return [
            q_p.tile([128, N], f16, tag=f"q{i}", name=f"q{b}{i}")
            for i in range(CP)
        ]

    def load_q(b, q, half):
        # on the sync (SP HWDGE) ring so FIFO order keeps these BEHIND the
        # critical qT transposes — the scheduler would otherwise run them
        # early and starve MM1 on the serialized DMA path.  Column-halved:
        # MM2's jpp-outer group order only needs half 0 to start.
        h = N // 2
        for i in range(CP):
            nc.sync.dma_start(
                out=q[i][:, half * h:(half + 1) * h],
                in_=x[b, 128 * i:128 * (i + 1), half * h:(half + 1) * h],
            )

    # identities (fp16 for warmup transposes, f32 for triangle recon)
    ident = const_p.tile([128, 128], f16)
    make_identity(nc, ident)
    ident_f = const_p.tile([128, 128], f32)
    make_identity(nc, ident_f)

    # HAM warmup: PE transposes of a memset scratch keep the PE busy while
    # the first transfers land, so the clock gate opens before real matmuls.
    warm_in = const_p.tile([128, 128], f16)
    nc.vector.memset(warm_in, 1.0)
    warm_ps = ps_p.tile([128, C], f16, tag="a7", name="warm_ps")
    for w in range(16):
        nc.tensor.transpose(
            warm_ps[:, 128 * (w % CP):128 * (w % CP + 1)], warm_in, ident)
    warm_sb = const_p.tile([128, C], f16)
    nc.vector.tensor_copy(out=warm_sb, in_=warm_ps)

    affs = {}

    def phase1_gen(b, qtg, base):
        aff = [aff_tile(base + i, f"aff{b}_{i}") for i in range(CP)]
        affs[b] = aff
        # strict upper triangle of the symmetric affinity: row block i
        # computes cols >= 128*i (free dims 512/384/256/128); lower
        # blocks reconstructed from their transposes afterwards.
        for k in range(KP):
            g, kk = divmod(k, KPG)
            qt = qtg[g]
            for i in range(CP):
                lo = 128 * i
                nc.tensor.matmul(
                    aff[i][:, lo:],
                    qt[:, kk, 128 * i:128 * (i + 1)],
                    qt[:, kk, lo:],
                    start=(k == 0),
                    stop=(k == KP - 1),
                )
            yield

    def recon_copies(b):
        # ACT copies of the upper-mirror blocks into SBUF staging
        tmps = []
        for (bi, bj) in TRI:
            tmp = tri_p.tile([128, 128], f32, tag="tri", name=f"tri{b}{bi}{bj}")
            nc.scalar.copy(
                out=tmp, in_=affs[b][bj][:, 128 * bi:128 * (bi + 1)])
            tmps.append(tmp)
        return tmps

    def recon_tr_gen(b, tmps):
        # PE transposes of the staged mirrors back into the lower triangle
        for (bi, bj), tmp in zip(TRI, tmps):
            nc.tensor.matmul(
                affs[b][bi][:, 128 * bj:128 * (bj + 1)],
                tmp, ident_f, is_transpose=True, skip_group_check=True,
            )
            yield

    def softmax_gen(b, out_attn, atg):
        # softmax(min-centered, negated), pre-scaled by gamma/Z; each
        # finished block immediately XBARs into the attnT layout.
        aff = affs[b]
        for i in range(CP):
            m = small_p.tile([128, 1], f32, tag="m")
            nc.vector.tensor_reduce(
                out=m, in_=aff[i], op=mybir.AluOpType.min, axis=mybir.AxisListType.X
            )
            a_t = attn_p.tile([128, C], f16, tag="a_t", name="a_t")
            z = small_p.tile([128, 1], f32, tag="z")
            nc.scalar.activation(
                out=a_t, in_=aff[i], func=mybir.ActivationFunctionType.Exp,
                bias=m, scale=-1.0, accum_out=z,
            )
            rz = small_p.tile([128, 1], f32, tag="rz")
            nc.vector.reciprocal(out=rz, in_=z)
            g = small_p.tile([128, 1], f32, tag="grz", name="grz")
            nc.vector.tensor_scalar_mul(out=g, in0=rz, scalar1=gamma_sb)
            nc.vector.tensor_scalar_mul(out=a_t, in0=a_t, scalar1=g)
            out_attn.append(a_t)
            nc.sync.dma_start(
                out=atg[:, :, 128 * i:128 * (i + 1)],
                in_=a_t,
                transpose=True,
            )
            yield

    def mm2_gen(b, q, atg, base, last=False):
        # MM2 + epilogue (mixed-dtype DVE add from PSUM, fp16 y DMA).
        # jpp-outer: the first four groups touch only column-half 0 of q,
        # which the halved loads deliver first.
        for jpp in range(2):
            for i in range(CP):
                o = outsb_p.tile([128, 2048], f16, tag="o", name=f"o{b}{i}{jpp}")
                # final tiles of the kernel stream y out per 512-col
                # slice so the tail drain is one small DMA, not 512KB
                fine = last and jpp == 1 and i >= CP - 2
                for jh in range(4):
                    j = 4 * jpp + jh
                    po = ps_p.tile([128, 512], f32, tag=f"a{base + jh}",
                                   name=f"po{b}{i}{j}")
                    for kd in range(CP):
                        nc.tensor.matmul(
                            po,
                            atg[:, kd, 128 * i:128 * (i + 1)],
                            q[kd][:, 512 * j:512 * (j + 1)],
                            start=(kd == 0),
                            stop=(kd == CP - 1),
                        )
                    nc.vector.tensor_add(
                        out=o[:, 512 * jh:512 * (jh + 1)], in0=po,
                        in1=q[i][:, 512 * j:512 * (j + 1)],
                    )
                    if fine:
                        nc.scalar.dma_start(
                            out=y[b, 128 * i:128 * (i + 1),
                                  512 * j:512 * (j + 1)],
                            in_=o[:, 512 * jh:512 * (jh + 1)],
                        )
                if not fine:
                    nc.scalar.dma_start(
                        out=y[b, 128 * i:128 * (i + 1),
                              2048 * jpp:2048 * (jpp + 1)],
                        in_=o,
                    )
                yield

    def drain(g):
        for _ in g:
            pass

    def step(g, n):
        for _ in range(n):
            if next(g, StopIteration) is StopIteration:
                return

    # ---- emission schedule ----
    # First qT transfers for b0 (g0 split so MM1 starts early), then the
    # rest of b0's groups; q loads deliberately come later.  gamma comes
    # in AS a transpose (host pre-tiles it to [16,128] f16): a plain DMA
    # anywhere near the qT stream serializes against every transpose via
    # the XBAR-deadlock guard and stalls MM1 by multiple microseconds.
    qt0 = xbar_qT(0, fine=True)
    gamma_16 = const_p.tile([128, 16], f16)
    nc.sync.dma_start(out=gamma_16, in_=gamma, transpose=True)
    gamma_sb = const_p.tile([128, 1], f32)
    nc.vector.tensor_copy(out=gamma_sb, in_=gamma_16[:, 0:1])

    g_mm0 = phase1_gen(0, qt0, base=0)
    drain(g_mm0)

    qt1 = xbar_qT(1)
    q0 = alloc_q(0)

    # recon(b0) copies run on ACT as soon as aff(b0) closes; transposes,
    # softmax(b0) and its attnT XBARs interleave into MM1(b1)'s stream.
    # q loads are emitted only AFTER the attnT XBARs: the serialized DMA
    # path services in emission order and MM2(b0) needs attnT first.
    tmps0 = recon_copies(0)
    atg0 = attnT_p.tile([128, CP, 512], f16, tag="at", name="at0")
    g_mm1 = phase1_gen(1, qt1, base=4)
    step(g_mm1, 2)
    tr0 = recon_tr_gen(0, tmps0)
    sm0 = softmax_gen(0, attns0 := [], atg0)
    step(sm0, 1)          # block 0 needs no recon
    step(g_mm1, 2)
    step(tr0, 1)          # (1,0)
    step(sm0, 1)
    step(g_mm1, 2)
    step(tr0, 2)          # (2,0) (2,1)
    step(sm0, 1)
    step(g_mm1, 2)
    step(tr0, 3)          # (3,*)
    step(sm0, 1)
    load_q(0, q0, 0)
    load_q(0, q0, 1)
    q1 = alloc_q(1)
    load_q(1, q1, 0)
    load_q(1, q1, 1)
    drain(g_mm1)

    # recon(b1)/softmax(b1)/attnT(b1) hide under MM2(b0)
    tmps1 = recon_copies(1)
    atg1 = attnT_p.tile([128, CP, 512], f16, tag="at", name="at1")
    g_mm2_0 = mm2_gen(0, q0, atg0, base=4)
    step(g_mm2_0, 1)
    tr1 = recon_tr_gen(1, tmps1)
    sm1 = softmax_gen(1, attns1 := [], atg1)
    step(sm1, 1)
    step(g_mm2_0, 1)
    step(tr1, 1)
    step(sm1, 1)
    step(g_mm2_0, 1)
    step(tr1, 2)
    step(sm1, 1)
    step(g_mm2_0, 1)
    step(tr1, 3)
    step(sm1, 1)
    drain(g_mm2_0)

    drain(mm2_gen(1, q1, atg1, base=0, last=True))

    for p in reversed(list(pools.values())):
        p.release()


_NC_CACHE = {}


def build_kernel(bpc=BPC, repeat=1):
    key = (bpc, repeat)
    if key in _NC_CACHE:
        return _NC_CACHE[key]
    assert bpc == BPC
    nc = bacc.Bacc("TRN2", target_bir_lowering=False, debug=False, num_devices=1)
    x = nc.dram_tensor("x", [bpc, C, N], f16, kind="ExternalInput").ap()
    # gamma pre-tiled to [16, 128] f16 by the host so it can ride the
    # XBAR-transpose path (see _build_body)
    gamma = nc.dram_tensor("gamma", [16, 128], f16, kind="ExternalInput").ap()
    y = nc.dram_tensor("y", [bpc, C, N], f16, kind="ExternalOutput").ap()
    with tile.TileContext(nc) as tc:
        for _ in range(repeat):
            _build_body(nc, tc, x, gamma, y)
    nc.compile()
    _NC_CACHE[key] = nc
    return nc


def make_in_maps(x, gamma):
    """Host-side prep: cast x to fp16 and shard over cores."""
    x = np.ascontiguousarray(x, dtype=np.float32).reshape(B, C, N)
    x16 = x.astype(np.float16)
    g16 = np.full((16, 128), np.float16(np.asarray(gamma).reshape(-1)[0]))
    return [
        {"x": x16[i * BPC:(i + 1) * BPC], "gamma": g16} for i in range(NCORES)
    ]


def run(x, gamma, trace=False):
    """x: [B, C, H, W] f32, gamma: [1] f32 -> ([B, C, H, W] f32, results)"""
    nc = build_kernel()
    in_maps = make_in_maps(x, gamma)
    res = run_bass_kernel_spmd(nc, in_maps, core_ids=list(range(NCORES)),
                               trace=trace)
    out = np.concatenate(
        [res.results[i]["y"].astype(np.float32) for i in range(NCORES)], axis=0)
    return out.reshape(B, C, H, W), res


def kernel(x, gamma):
    out, _ = run(x, gamma)
    return out


# revision 3
# speedup vs baseline: 1.1905x; 1.1905x over previous
"""Channel-attention block kernel for Trainium2 (8 NeuronCores, SPMD) — v2.

Reference (per batch b):
    q = x[b]                       # [C=512, N=4096]  (N = H*W)
    aff = q @ q.T                  # [512, 512]
    attn = softmax(rowmax(aff) - aff, axis=-1)
         = exp(rowmin(aff) - aff) / rowsum(...)
    out[b] = gamma * (attn @ x[b]) + x[b]

v2 over the baseline (183us):
  - y stored fp16 on device (host casts back to f32): halves output DMA.
  - aff(b0) uses PSUM banks 0-3, aff(b1) banks 4-7; MM2 po tiles reuse
    the other batch's banks.  Removes every WAR stall between batches.
  - qT XBAR transposes are one whole-C transfer per 1024-col group
    (first group split finer so MM1 starts early); q loads for MM2 are
    emitted AFTER the transposes so they don't starve MM1 on the
    serialized DMA path.
  - recon/softmax/attnT emission interleaved into the next phase's PE
    stream so the PE never idles between MM1(b0)/MM1(b1)/MM2(b0)/MM2(b1).
"""

import numpy as np

import concourse.bacc as bacc
import concourse.tile as tile
from concourse import mybir
from concourse.bass_utils import run_bass_kernel_spmd
from concourse.masks import make_identity

B, C, H, W = 16, 512, 64, 64
N = H * W            # 4096
NCORES = 8
BPC = B // NCORES    # batches per core
CP = C // 128        # 4 channel blocks
KP = N // 128        # 32 n-chunks
KG = 4               # qT transpose groups per batch
KPG = KP // KG       # 8 k-chunks per group

f32 = mybir.dt.float32
f16 = mybir.dt.float16

# lower-triangle blocks reconstructed from their upper mirrors, ordered
# so softmax rows complete in order 1,2,3 (row 0 needs no recon)
TRI = [(1, 0), (2, 0), (2, 1), (3, 0), (3, 1), (3, 2)]


def _setup_pools(nc, tc):
    """Pools + constants shared by every repeat body: hoisting them out
    of the body removes the per-body alloc/release barriers so repeat
    bodies software-pipeline (tags ring across bodies with WAR deps)."""
    pools = {}

    def pool(name, bufs, space="SBUF"):
        pools[name] = tc.alloc_tile_pool(name=name, bufs=bufs, space=space)
        return pools[name]

    ctx = {
        "pools": pools,
        "q_p": pool("q", BPC),            # [128, N] f16 per channel block
        "qt_p": pool("qt", BPC),          # [128, KPG, 512] f16 qT groups
        "tri_p": pool("tri", 6),          # [128, 128] f32 recon staging
        "attn_p": pool("attn", 2 * CP),   # [128, C] f16
        "attnT_p": pool("attnT", 2),      # [128, CP, 512] f16
        "outsb_p": pool("outsb", 6),      # [128, 2048] f16
        "small_p": pool("small", 8),      # [128, 1] f32
        "const_p": pool("const", 1),
        "ps_p": pool("ps", 1, space="PSUM"),  # tags a0..a7 -> the 8 banks
    }
    const_p = ctx["const_p"]
    # identities (fp16 for warmup transposes, f32 for triangle recon)
    ident = const_p.tile([128, 128], f16)
    make_identity(nc, ident)
    ident_f = const_p.tile([128, 128], f32)
    make_identity(nc, ident_f)
    warm_in = const_p.tile([128, 128], f16)
    nc.vector.memset(warm_in, 1.0)
    ctx.update(ident=ident, ident_f=ident_f, warm_in=warm_in)
    return ctx


def _build_body(nc, tc, x, gamma, y, ctx, first):
    q_p = ctx["q_p"]
    qt_p = ctx["qt_p"]
    tri_p = ctx["tri_p"]
    attn_p = ctx["attn_p"]
    attnT_p = ctx["attnT_p"]
    outsb_p = ctx["outsb_p"]
    small_p = ctx["small_p"]
    const_p = ctx["const_p"]
    ps_p = ctx["ps_p"]
    ident = ctx["ident"]
    ident_f = ctx["ident_f"]
    warm_in = ctx["warm_in"]

    def aff_tile(t, name):
        return ps_p.tile([128, C], f32, tag=f"a{t}", name=name)

    # qT via DMA XBAR straight from DRAM: group g transposes
    # x[b, :, 1024g:1024(g+1)] ([512c, 1024n]) into qt[g] = [128n, 8k, 512c].
    # XBAR transposes stay on the SP ring only (concurrent transposes on
    # both HWDGE rings corrupt nondeterministically on HW).
    def alloc_qt(b):
        return [
            qt_p.tile([128, KPG, 512], f16, tag=f"qt{g}", name=f"qt{b}{g}")
            for g in range(KG)
        ]

    def emit_qT(b, qtg, g, splits=((0, KPG),)):
        for (k0, k1) in splits:
            nc.sync.dma_start(
                out=qtg[g][:, k0:k1, :],
                in_=x[b, :, 1024 * g + 128 * k0:1024 * g + 128 * k1],
                transpose=True,
            )

    def xbar_qT(b, fine=False):
        # fine=True (critical-path batch 0): half-group transfers so the
        # chunk-arrival curve stays ahead of MM1's consumption rate
        qtg = alloc_qt(b)
        for g in range(KG):
            if fine:
                splits = ((0, 2), (2, 4), (4, 8)) if g == 0 else ((0, 4), (4, 8))
            else:
                splits = ((0, KPG),)
            emit_qT(b, qtg, g, splits)
        return qtg

    def alloc_q(b):
        return [
            q_p.tile([128, N], f16, tag=f"q{i}", name=f"q{b}{i}")
            for i in range(CP)
        ]

    def load_q(b, q, half):
        # on the sync (SP HWDGE) ring so FIFO order keeps these BEHIND the
        # critical qT transposes — the scheduler would otherwise run them
        # early and starve MM1 on the serialized DMA path.  Column-halved:
        # MM2's jpp-outer group order only needs half 0 to start.
        h = N // 2
        for i in range(CP):
            nc.sync.dma_start(
                out=q[i][:, half * h:(half + 1) * h],
                in_=x[b, 128 * i:128 * (i + 1), half * h:(half + 1) * h],
            )

    affs = {}

    def phase1_gen(b, qtg, base):
        aff = [aff_tile(base + i, f"aff{b}_{i}") for i in range(CP)]
        affs[b] = aff
        # strict upper triangle of the symmetric affinity: row block i
        # computes cols >= 128*i (free dims 512/384/256/128); lower
        # blocks reconstructed from their transposes afterwards.
        for k in range(KP):
            g, kk = divmod(k, KPG)
            qt = qtg[g]
            for i in range(CP):
                lo = 128 * i
                nc.tensor.matmul(
                    aff[i][:, lo:],
                    qt[:, kk, 128 * i:128 * (i + 1)],
                    qt[:, kk, lo:],
                    start=(k == 0),
                    stop=(k == KP - 1),
                )
            yield

    def recon_copies(b):
        # ACT copies of the upper-mirror blocks into SBUF staging
        tmps = []
        for (bi, bj) in TRI:
            tmp = tri_p.tile([128, 128], f32, tag="tri", name=f"tri{b}{bi}{bj}")
            nc.scalar.copy(
                out=tmp, in_=affs[b][bj][:, 128 * bi:128 * (bi + 1)])
            tmps.append(tmp)
        return tmps

    def recon_tr_gen(b, tmps):
        # PE transposes of the staged mirrors back into the lower triangle
        for (bi, bj), tmp in zip(TRI, tmps):
            nc.tensor.matmul(
                affs[b][bi][:, 128 * bj:128 * (bj + 1)],
                tmp, ident_f, is_transpose=True, skip_group_check=True,
            )
            yield

    def softmax_gen(b, out_attn, atg):
        # softmax(min-centered, negated), pre-scaled by gamma/Z; each
        # finished block immediately XBARs into the attnT layout.
        aff = affs[b]
        for i in range(CP):
            m = small_p.tile([128, 1], f32, tag="m")
            nc.vector.tensor_reduce(
                out=m, in_=aff[i], op=mybir.AluOpType.min, axis=mybir.AxisListType.X
            )
            a_t = attn_p.tile([128, C], f16, tag="a_t", name="a_t")
            z = small_p.tile([128, 1], f32, tag="z")
            nc.scalar.activation(
                out=a_t, in_=aff[i], func=mybir.ActivationFunctionType.Exp,
                bias=m, scale=-1.0, accum_out=z,
            )
            rz = small_p.tile([128, 1], f32, tag="rz")
            nc.vector.reciprocal(out=rz, in_=z)
            g = small_p.tile([128, 1], f32, tag="grz", name="grz")
            nc.vector.tensor_scalar_mul(out=g, in0=rz, scalar1=gamma_sb)
            nc.vector.tensor_scalar_mul(out=a_t, in0=a_t, scalar1=g)
            out_attn.append(a_t)
            nc.sync.dma_start(
                out=atg[:, :, 128 * i:128 * (i + 1)],
                in_=a_t,
                transpose=True,
            )
            yield

    def mm2_gen(b, q, atg, base, last=False):
        # MM2 + epilogue (mixed-dtype DVE add from PSUM, fp16 y DMA).
        # jpp-outer: the first four groups touch only column-half 0 of q,
        # which the halved loads deliver first.
        for jpp in range(2):
            for i in range(CP):
                o = outsb_p.tile([128, 2048], f16, tag="o", name=f"o{b}{i}{jpp}")
                # final tiles of the kernel stream y out per 512-col
                # slice so the tail drain is one small DMA, not 512KB
                fine = last and jpp == 1 and i >= CP - 2
                for jh in range(4):
                    j = 4 * jpp + jh
                    po = ps_p.tile([128, 512], f32, tag=f"a{base + jh}",
                                   name=f"po{b}{i}{j}")
                    for kd in range(CP):
                        nc.tensor.matmul(
                            po,
                            atg[:, kd, 128 * i:128 * (i + 1)],
                            q[kd][:, 512 * j:512 * (j + 1)],
                            start=(kd == 0),
                            stop=(kd == CP - 1),
                        )
                    nc.vector.tensor_add(
                        out=o[:, 512 * jh:512 * (jh + 1)], in0=po,
                        in1=q[i][:, 512 * j:512 * (j + 1)],
                    )
                    if fine:
                        nc.scalar.dma_start(
                            out=y[b, 128 * i:128 * (i + 1),
                                  512 * j:512 * (j + 1)],
                            in_=o[:, 512 * jh:512 * (jh + 1)],
                        )
                if not fine:
                    nc.scalar.dma_start(
                        out=y[b, 128 * i:128 * (i + 1),
                              2048 * jpp:2048 * (jpp + 1)],
                        in_=o,
                    )
                yield

    def drain(g):
        for _ in g:
            pass

    def step(g, n):
        for _ in range(n):
            if next(g, StopIteration) is StopIteration:
                return

    # ---- emission schedule ----
    # First qT transfers for b0 (g0 split so MM1 starts early), then the
    # rest of b0's groups; q loads deliberately come later.  gamma comes
    # in AS a transpose (host pre-tiles it to [16,128] f16): a plain DMA
    # anywhere near the qT stream serializes against every transpose via
    # the XBAR-deadlock guard and stalls MM1 by multiple microseconds.
    qt0 = xbar_qT(0, fine=first)
    if first:
        # HAM warmup: PE transposes of a memset scratch keep the PE busy
        # while the first transfers land, so the clock gate opens before
        # real matmuls.  Later repeat bodies arrive with the PE warm.
        warm_ps = ps_p.tile([128, C], f16, tag="a7", name="warm_ps")
        for w in range(16):
            nc.tensor.transpose(
                warm_ps[:, 128 * (w % CP):128 * (w % CP + 1)], warm_in, ident)
        warm_sb = const_p.tile([128, C], f16)
        nc.vector.tensor_copy(out=warm_sb, in_=warm_ps)
        gamma_16 = const_p.tile([128, 16], f16)
        nc.sync.dma_start(out=gamma_16, in_=gamma, transpose=True)
        gamma_sb = const_p.tile([128, 1], f32)
        nc.vector.tensor_copy(out=gamma_sb, in_=gamma_16[:, 0:1])
        ctx["gamma_sb"] = gamma_sb
    gamma_sb = ctx["gamma_sb"]

    g_mm0 = phase1_gen(0, qt0, base=0)
    drain(g_mm0)

    qt1 = xbar_qT(1)
    q0 = alloc_q(0)

    # recon(b0) copies run on ACT as soon as aff(b0) closes; transposes,
    # softmax(b0) and its attnT XBARs interleave into MM1(b1)'s stream.
    # q loads are emitted only AFTER the attnT XBARs: the serialized DMA
    # path services in emission order and MM2(b0) needs attnT first.
    tmps0 = recon_copies(0)
    atg0 = attnT_p.tile([128, CP, 512], f16, tag="at", name="at0")
    g_mm1 = phase1_gen(1, qt1, base=4)
    step(g_mm1, 2)
    tr0 = recon_tr_gen(0, tmps0)
    sm0 = softmax_gen(0, attns0 := [], atg0)
    step(sm0, 1)          # block 0 needs no recon
    step(g_mm1, 2)
    step(tr0, 1)          # (1,0)
    step(sm0, 1)
    step(g_mm1, 2)
    step(tr0, 2)          # (2,0) (2,1)
    step(sm0, 1)
    step(g_mm1, 2)
    step(tr0, 3)          # (3,*)
    step(sm0, 1)
    load_q(0, q0, 0)
    load_q(0, q0, 1)
    q1 = alloc_q(1)
    load_q(1, q1, 0)
    load_q(1, q1, 1)
    drain(g_mm1)

    # recon(b1)/softmax(b1)/attnT(b1) hide under MM2(b0)
    tmps1 = recon_copies(1)
    atg1 = attnT_p.tile([128, CP, 512], f16, tag="at", name="at1")
    g_mm2_0 = mm2_gen(0, q0, atg0, base=4)
    step(g_mm2_0, 1)
    tr1 = recon_tr_gen(1, tmps1)
    sm1 = softmax_gen(1, attns1 := [], atg1)
    step(sm1, 1)
    step(g_mm2_0, 1)
    step(tr1, 1)
    step(sm1, 1)
    step(g_mm2_0, 1)
    step(tr1, 2)
    step(sm1, 1)
    step(g_mm2_0, 1)
    step(tr1, 3)
    step(sm1, 1)
    drain(g_mm2_0)

    drain(mm2_gen(1, q1, atg1, base=0, last=True))


_NC_CACHE = {}


def build_kernel(bpc=BPC, repeat=1):
    key = (bpc, repeat)
    if key in _NC_CACHE:
        return _NC_CACHE[key]
    assert bpc == BPC
    nc = bacc.Bacc("TRN2", target_bir_lowering=False, debug=False, num_devices=1)
    x = nc.dram_tensor("x", [bpc, C, N], f16, kind="ExternalInput").ap()
    # gamma pre-tiled to [16, 128] f16 by the host so it can ride the
    # XBAR-transpose path (see _build_body)
    gamma = nc.dram_tensor("gamma", [16, 128], f16, kind="ExternalInput").ap()
    y = nc.dram_tensor("y", [bpc, C, N], f16, kind="ExternalOutput").ap()
    with tile.TileContext(nc) as tc:
        ctx = _setup_pools(nc, tc)
        for r in range(repeat):
            _build_body(nc, tc, x, gamma, y, ctx, first=(r == 0))
        for p in reversed(list(ctx["pools"].values())):
            p.release()
    nc.compile()
    _NC_CACHE[key] = nc
    return nc


def make_in_maps(x, gamma):
    """Host-side prep: cast x to fp16 and shard over cores."""
    x = np.ascontiguousarray(x, dtype=np.float32).reshape(B, C, N)
    x16 = x.astype(np.float16)
    g16 = np.full((16, 128), np.float16(np.asarray(gamma).reshape(-1)[0]))
    return [
        {"x": x16[i * BPC:(i + 1) * BPC], "gamma": g16} for i in range(NCORES)
    ]


def run(x, gamma, trace=False):
    """x: [B, C, H, W] f32, gamma: [1] f32 -> ([B, C, H, W] f32, results)"""
    nc = build_kernel()
    in_maps = make_in_maps(x, gamma)
    res = run_bass_kernel_spmd(nc, in_maps, core_ids=list(range(NCORES)),
                               trace=trace)
    out = np.concatenate(
        [res.results[i]["y"].astype(np.float32) for i in range(NCORES)], axis=0)
    return out.reshape(B, C, H, W), res


def kernel(x, gamma):
    out, _ = run(x, gamma)
    return out


# revision 5
# speedup vs baseline: 1.3661x; 1.1476x over previous
"""Channel-attention block kernel for Trainium2 (8 NeuronCores, SPMD).

Reference (per batch b):
    q = x[b]                       # [C=512, N=4096]  (N = H*W)
    aff = q @ q.T                  # [512, 512]
    attn = softmax(rowmax(aff) - aff, axis=-1)
         = exp(rowmin(aff) - aff) / rowsum(...)   (max-shift cancels)
    out[b] = gamma * (attn @ x[b]) + x[b]

Data-parallel over B: 2 batches per core.  fp16 everywhere on chip
(rel err ~5e-4 vs the 2e-2 budget); aff accumulates f32 in PSUM.

Key design points (vs the 183us predecessor; measured 79-101us):
  - y stored fp16 on device (host casts back to f32): halves output DMA.
  - aff(b0) owns PSUM banks 0-3, aff(b1) banks 4-7; MM2's po tiles reuse
    the opposite batch's banks.  No WAR stalls between batches, so
    recon/softmax/attnT of one batch hide under the other's matmuls.
  - All data transposes ride the DMA XBAR (PE transpose-mode is slow and
    does not count as PE-busy for the HAM clock): qT groups straight
    from DRAM as whole-C transfers (first group split finer so MM1
    starts early), attnT from SBUF per softmax block.
  - Emission order = service order on the serialized DMA path, so
    ordering is load-bearing: qT(b0), qT(b1), attnT(b0), q loads
    (column-halved; MM2 is jpp-outer so half 0 suffices to start), y
    writes.  gamma is host-pre-tiled to [16,128] f16 and loaded AS a
    transpose: a plain DMA near the qT stream serializes against every
    transpose via the XBAR-deadlock guard and stalls MM1.
  - MM1 computes only the strict upper triangle of the symmetric
    affinity (free dims 512/384/256/128); lower blocks are rebuilt by
    ACT copy + PE transpose, interleaved into the next phase.
  - Pools/constants are allocated once and shared by all repeat bodies,
    so bodies software-pipeline (no per-body alloc/release barriers);
    HAM warmup + gamma load only in the first body.
"""

import numpy as np

import concourse.bacc as bacc
import concourse.tile as tile
from concourse import mybir
from concourse.bass_utils import run_bass_kernel_spmd
from concourse.masks import make_identity

B, C, H, W = 16, 512, 64, 64
N = H * W            # 4096
NCORES = 8
BPC = B // NCORES    # batches per core
CP = C // 128        # 4 channel blocks
KP = N // 128        # 32 n-chunks
KG = 4               # qT transpose groups per batch
KPG = KP // KG       # 8 k-chunks per group

f32 = mybir.dt.float32
f16 = mybir.dt.float16

# lower-triangle blocks reconstructed from their upper mirrors, ordered
# so softmax rows complete in order 1,2,3 (row 0 needs no recon)
TRI = [(1, 0), (2, 0), (2, 1), (3, 0), (3, 1), (3, 2)]


def _setup_pools(nc, tc):
    """Pools + constants shared by every repeat body: hoisting them out
    of the body removes the per-body alloc/release barriers so repeat
    bodies software-pipeline (tags ring across bodies with WAR deps)."""
    pools = {}

    def pool(name, bufs, space="SBUF"):
        pools[name] = tc.alloc_tile_pool(name=name, bufs=bufs, space=space)
        return pools[name]

    ctx = {
        "pools": pools,
        "q_p": pool("q", BPC),            # [128, N] f16 per channel block
        "qt_p": pool("qt", BPC),          # [128, KPG, 512] f16 qT groups
        "tri_p": pool("tri", 6),          # [128, 128] f32 recon staging
        "attn_p": pool("attn", 2 * CP),   # [128, C] f16
        "attnT_p": pool("attnT", 2),      # [128, CP, 512] f16
        "outsb_p": pool("outsb", 6),      # [128, 2048] f16
        "small_p": pool("small", 8),      # [128, 1] f32
        "const_p": pool("const", 1),
        "ps_p": pool("ps", 1, space="PSUM"),  # tags a0..a7 -> the 8 banks
    }
    const_p = ctx["const_p"]
    # identities (fp16 for warmup transposes, f32 for triangle recon)
    ident = const_p.tile([128, 128], f16)
    make_identity(nc, ident)
    ident_f = const_p.tile([128, 128], f32)
    make_identity(nc, ident_f)
    warm_in = const_p.tile([128, 128], f16)
    nc.vector.memset(warm_in, 1.0)
    ctx.update(ident=ident, ident_f=ident_f, warm_in=warm_in)
    return ctx


def _build_body(nc, tc, x, gamma, y, ctx, first):
    q_p = ctx["q_p"]
    qt_p = ctx["qt_p"]
    tri_p = ctx["tri_p"]
    attn_p = ctx["attn_p"]
    attnT_p = ctx["attnT_p"]
    outsb_p = ctx["outsb_p"]
    small_p = ctx["small_p"]
    const_p = ctx["const_p"]
    ps_p = ctx["ps_p"]
    ident = ctx["ident"]
    ident_f = ctx["ident_f"]
    warm_in = ctx["warm_in"]

    def aff_tile(t, name):
        return ps_p.tile([128, C], f32, tag=f"a{t}", name=name)

    # qT via DMA XBAR straight from DRAM: group g transposes
    # x[b, :, 1024g:1024(g+1)] ([512c, 1024n]) into qt[g] = [128n, 8k, 512c].
    # XBAR transposes stay on the SP ring only (concurrent transposes on
    # both HWDGE rings corrupt nondeterministically on HW).
    def alloc_qt(b):
        return [
            qt_p.tile([128, KPG, 512], f16, tag=f"qt{g}", name=f"qt{b}{g}")
            for g in range(KG)
        ]

    def emit_qT(b, qtg, g, splits=((0, KPG),)):
        for (k0, k1) in splits:
            nc.sync.dma_start(
                out=qtg[g][:, k0:k1, :],
                in_=x[b, :, 1024 * g + 128 * k0:1024 * g + 128 * k1],
                transpose=True,
            )

    def xbar_qT(b, fine=False):
        # fine=True (critical-path batch 0): half-group transfers so the
        # chunk-arrival curve stays ahead of MM1's consumption rate
        qtg = alloc_qt(b)
        for g in range(KG):
            if fine:
                splits = ((0, 2), (2, 4), (4, 8)) if g == 0 else ((0, 4), (4, 8))
            else:
                splits = ((0, KPG),)
            emit_qT(b, qtg, g, splits)
        return qtg

    def alloc_q(b):
        return [
            q_p.tile([128, N], f16, tag=f"q{i}", name=f"q{b}{i}")
            for i in range(CP)
        ]

    def load_q(b, q, half):
        # on the sync (SP HWDGE) ring so FIFO order keeps these BEHIND the
        # critical qT transposes — the scheduler would otherwise run them
        # early and starve MM1 on the serialized DMA path.  Column-halved:
        # MM2's jpp-outer group order only needs half 0 to start.
        h = N // 2
        for i in range(CP):
            nc.sync.dma_start(
                out=q[i][:, half * h:(half + 1) * h],
                in_=x[b, 128 * i:128 * (i + 1), half * h:(half + 1) * h],
            )

    affs = {}

    def phase1_gen(b, qtg, base):
        aff = [aff_tile(base + i, f"aff{b}_{i}") for i in range(CP)]
        affs[b] = aff
        # strict upper triangle of the symmetric affinity: row block i
        # computes cols >= 128*i (free dims 512/384/256/128); lower
        # blocks reconstructed from their transposes afterwards.
        for k in range(KP):
            g, kk = divmod(k, KPG)
            qt = qtg[g]
            for i in range(CP):
                lo = 128 * i
                nc.tensor.matmul(
                    aff[i][:, lo:],
                    qt[:, kk, 128 * i:128 * (i + 1)],
                    qt[:, kk, lo:],
                    start=(k == 0),
                    stop=(k == KP - 1),
                )
            yield

    def recon_copies(b):
        # ACT copies of the upper-mirror blocks into SBUF staging
        tmps = []
        for (bi, bj) in TRI:
            tmp = tri_p.tile([128, 128], f32, tag="tri", name=f"tri{b}{bi}{bj}")
            nc.scalar.copy(
                out=tmp, in_=affs[b][bj][:, 128 * bi:128 * (bi + 1)])
            tmps.append(tmp)
        return tmps

    def recon_tr_gen(b, tmps):
        # PE transposes of the staged mirrors back into the lower triangle
        for (bi, bj), tmp in zip(TRI, tmps):
            nc.tensor.matmul(
                affs[b][bi][:, 128 * bj:128 * (bj + 1)],
                tmp, ident_f, is_transpose=True, skip_group_check=True,
            )
            yield

    def softmax_gen(b, out_attn, atg):
        # softmax(min-centered, negated), pre-scaled by gamma/Z; each
        # finished block immediately XBARs into the attnT layout.
        aff = affs[b]
        for i in range(CP):
            m = small_p.tile([128, 1], f32, tag="m")
            nc.vector.tensor_reduce(
                out=m, in_=aff[i], op=mybir.AluOpType.min, axis=mybir.AxisListType.X
            )
            a_t = attn_p.tile([128, C], f16, tag="a_t", name="a_t")
            z = small_p.tile([128, 1], f32, tag="z")
            nc.scalar.activation(
                out=a_t, in_=aff[i], func=mybir.ActivationFunctionType.Exp,
                bias=m, scale=-1.0, accum_out=z,
            )
            rz = small_p.tile([128, 1], f32, tag="rz")
            nc.vector.reciprocal(out=rz, in_=z)
            g = small_p.tile([128, 1], f32, tag="grz", name="grz")
            nc.vector.tensor_scalar_mul(out=g, in0=rz, scalar1=gamma_sb)
            nc.vector.tensor_scalar_mul(out=a_t, in0=a_t, scalar1=g)
            out_attn.append(a_t)
            nc.sync.dma_start(
                out=atg[:, :, 128 * i:128 * (i + 1)],
                in_=a_t,
                transpose=True,
            )
            yield

    def mm2_gen(b, q, atg, base, last=False):
        # MM2 + epilogue (mixed-dtype DVE add from PSUM, fp16 y DMA).
        # jpp-outer: the first four groups touch only column-half 0 of q,
        # which the halved loads deliver first.
        for jpp in range(2):
            for i in range(CP):
                o = outsb_p.tile([128, 2048], f16, tag="o", name=f"o{b}{i}{jpp}")
                # final tiles of the kernel stream y out per 512-col
                # slice so the tail drain is one small DMA, not 512KB
                fine = last and jpp == 1 and i >= CP - 2
                for jh in range(4):
                    j = 4 * jpp + jh
                    po = ps_p.tile([128, 512], f32, tag=f"a{base + jh}",
                                   name=f"po{b}{i}{j}")
                    for kd in range(CP):
                        nc.tensor.matmul(
                            po,
                            atg[:, kd, 128 * i:128 * (i + 1)],
                            q[kd][:, 512 * j:512 * (j + 1)],
                            start=(kd == 0),
                            stop=(kd == CP - 1),
                        )
                    nc.vector.tensor_add(
                        out=o[:, 512 * jh:512 * (jh + 1)], in0=po,
                        in1=q[i][:, 512 * j:512 * (j + 1)],
                    )
                    if fine:
                        nc.scalar.dma_start(
                            out=y[b, 128 * i:128 * (i + 1),
                                  512 * j:512 * (j + 1)],
                            in_=o[:, 512 * jh:512 * (jh + 1)],
                        )
                if not fine:
                    nc.scalar.dma_start(
                        out=y[b, 128 * i:128 * (i + 1),
                              2048 * jpp:2048 * (jpp + 1)],
                        in_=o,
                    )
                yield

    def drain(g):
        for _ in g:
            pass

    def step(g, n):
        for _ in range(n):
            if next(g, StopIteration) is StopIteration:
                return

    # ---- emission schedule ----
    # First qT transfers for b0 (g0 split so MM1 starts early), then the
    # rest of b0's groups; q loads deliberately come later.  gamma comes
    # in AS a transpose (host pre-tiles it to [16,128] f16): a plain DMA
    # anywhere near the qT stream serializes against every transpose via
    # the XBAR-deadlock guard and stalls MM1 by multiple microseconds.
    qt0 = xbar_qT(0, fine=first)
    if first:
        # HAM warmup: real matmuls on a memset scratch keep the PE busy
        # while the first transfers land, so the clock gate opens before
        # the affinity matmuls (PE transpose-mode does NOT count as
        # PE-busy for the HAM, so transposes would leave the clock cold).
        warm_ps = ps_p.tile([128, C], f32, tag="a7", name="warm_ps")
        for w in range(16):
            # warm_in as both operands: only its memset gates the warmup,
            # not the (slower) identity construction on Pool
            nc.tensor.matmul(
                warm_ps[:, 128 * (w % CP):128 * (w % CP + 1)], warm_in,
                warm_in, start=True, stop=True)
        warm_sb = const_p.tile([128, C], f16)
        nc.vector.tensor_copy(out=warm_sb, in_=warm_ps)
        gamma_16 = const_p.tile([128, 16], f16)
        nc.sync.dma_start(out=gamma_16, in_=gamma, transpose=True)
        gamma_sb = const_p.tile([128, 1], f32)
        nc.vector.tensor_copy(out=gamma_sb, in_=gamma_16[:, 0:1])
        ctx["gamma_sb"] = gamma_sb
    gamma_sb = ctx["gamma_sb"]

    g_mm0 = phase1_gen(0, qt0, base=0)
    drain(g_mm0)

    qt1 = xbar_qT(1)
    q0 = alloc_q(0)

    # recon(b0) copies run on ACT as soon as aff(b0) closes; transposes,
    # softmax(b0) and its attnT XBARs interleave into MM1(b1)'s stream.
    # q loads are emitted only AFTER the attnT XBARs: the serialized DMA
    # path services in emission order and MM2(b0) needs attnT first.
    tmps0 = recon_copies(0)
    atg0 = attnT_p.tile([128, CP, 512], f16, tag="at", name="at0")
    g_mm1 = phase1_gen(1, qt1, base=4)
    step(g_mm1, 2)
    tr0 = recon_tr_gen(0, tmps0)
    sm0 = softmax_gen(0, attns0 := [], atg0)
    step(sm0, 1)          # block 0 needs no recon
    step(g_mm1, 2)
    step(tr0, 1)          # (1,0)
    step(sm0, 1)
    step(g_mm1, 2)
    step(tr0, 2)          # (2,0) (2,1)
    step(sm0, 1)
    step(g_mm1, 2)
    step(tr0, 3)          # (3,*)
    step(sm0, 1)
    load_q(0, q0, 0)
    load_q(0, q0, 1)
    q1 = alloc_q(1)
    load_q(1, q1, 0)
    load_q(1, q1, 1)
    drain(g_mm1)

    # recon(b1)/softmax(b1)/attnT(b1) hide under MM2(b0)
    tmps1 = recon_copies(1)
    atg1 = attnT_p.tile([128, CP, 512], f16, tag="at", name="at1")
    g_mm2_0 = mm2_gen(0, q0, atg0, base=4)
    step(g_mm2_0, 1)
    tr1 = recon_tr_gen(1, tmps1)
    sm1 = softmax_gen(1, attns1 := [], atg1)
    step(sm1, 1)
    step(g_mm2_0, 1)
    step(tr1, 1)
    step(sm1, 1)
    step(g_mm2_0, 1)
    step(tr1, 2)
    step(sm1, 1)
    step(g_mm2_0, 1)
    step(tr1, 3)
    step(sm1, 1)
    drain(g_mm2_0)

    drain(mm2_gen(1, q1, atg1, base=0, last=True))


_NC_CACHE = {}


def build_kernel(bpc=BPC, repeat=1):
    key = (bpc, repeat)
    if key in _NC_CACHE:
        return _NC_CACHE[key]
    assert bpc == BPC
    nc = bacc.Bacc("TRN2", target_bir_lowering=False, debug=False, num_devices=1)
    x = nc.dram_tensor("x", [bpc, C, N], f16, kind="ExternalInput").ap()
    # gamma pre-tiled to [16, 128] f16 by the host so it can ride the
    # XBAR-transpose path (see _build_body)
    gamma = nc.dram_tensor("gamma", [16, 128], f16, kind="ExternalInput").ap()
    y = nc.dram_tensor("y", [bpc, C, N], f16, kind="ExternalOutput").ap()
    with tile.TileContext(nc) as tc:
        ctx = _setup_pools(nc, tc)
        for r in range(repeat):
            _build_body(nc, tc, x, gamma, y, ctx, first=(r == 0))
        for p in reversed(list(ctx["pools"].values())):
            p.release()
    nc.compile()
    _NC_CACHE[key] = nc
    return nc


def make_in_maps(x, gamma):
    """Host-side prep: cast x to fp16 and shard over cores."""
    x = np.ascontiguousarray(x, dtype=np.float32).reshape(B, C, N)
    x16 = x.astype(np.float16)
    g16 = np.full((16, 128), np.float16(np.asarray(gamma).reshape(-1)[0]))
    return [
        {"x": x16[i * BPC:(i + 1) * BPC], "gamma": g16} for i in range(NCORES)
    ]


def run(x, gamma, trace=False):
    """x: [B, C, H, W] f32, gamma: [1] f32 -> ([B, C, H, W] f32, results)"""
    nc = build_kernel()
    in_maps = make_in_maps(x, gamma)
    res = run_bass_kernel_spmd(nc, in_maps, core_ids=list(range(NCORES)),
                               trace=trace)
    out = np.concatenate(
        [res.results[i]["y"].astype(np.float32) for i in range(NCORES)], axis=0)
    return out.reshape(B, C, H, W), res


def kernel(x, gamma):
    out, _ = run(x, gamma)
    return out
